# revision 7
# baseline (speedup 1.0000x reference)
"""FECAM layer Trainium2 kernel.

Reference computation (per batch element b, X = x[b] in R^{512x512}, layout [l, c]):
    xp   = X^T                                  # [c, l]
    freq = xp @ D^T                             # DCT-II along l      [c, k]
    sd   = LN(freq) * gamma + beta              # LayerNorm over k
    h    = relu(sd @ W1^T)                      # [c, 2C]
    fw   = sigmoid(h @ W2^T)                    # [c, k]
    fw   = LN(fw) * gamma + beta
    out  = (xp * fw)^T = X .* fw^T              # [l, c]  (natural layout)

Device strategy (data parallel, 16 batch elements per core x 8 cores):
  - freq computed as matmul(lhsT=x_b_tiles [l,c], rhs=D^T tiles [l,k]) -> [c,k] psum
  - LN1 stats via bn_stats/bn_aggr (free-axis k), z=(freq-mu)*rstd via tensor_scalar
  - gamma/beta of LN1 folded into fc1 weights on host:
        W1g[h,k] = w1[h,k]*gamma[k],  b1[h] = sum_k beta[k]*w1[h,k]
  - z transposed 128x128 via PE into zT [k,c]; fc1: hT = relu(W1g @ zT + b1) in [h,c]
  - fc2: fw = sigmoid(hT^T @ W2^T) computed as matmul(lhsT=w2T cols, rhs=hT) -> [c,k]
  - LN2 same trick; affine applied after PE transpose as per-partition scale/bias
  - final: out_tile = (z2T*gamma+beta) .* x_tile, DMA'd out in natural layout
All matmuls in float32r (full fp32 precision, 1 cycle/row at free dim >= 256).
"""

import sys

if "/opt/trn_rl_repo" not in sys.path:
    sys.path.insert(0, "/opt/trn_rl_repo")

import numpy as np

P = 128
C = 512          # channels == seq len == dct size
H = 1024         # hidden
CT = C // P      # 4 c-tiles
KT = C // P      # 4 k-tiles
HT = H // P      # 8 h-tiles
EPS = 1e-6
N_CORES = 8
B_FULL = 128

_NC_CACHE: dict = {}

# matmul input dtype: "f32r" (fast, full fp32 bits) or "f32" (4x slower, safe)
MM_MODE = "f32r"


def _build(nb: int):
    import concourse.bass as bass
    from concourse import bacc
    import concourse.mybir as mybir
    from concourse.tile import TileContext
    from concourse.masks import make_identity

    f32 = mybir.dt.float32
    f32r = mybir.dt.float32r
    Relu = mybir.ActivationFunctionType.Relu
    Ln = mybir.ActivationFunctionType.Ln
    Exp = mybir.ActivationFunctionType.Exp
    Ident = mybir.ActivationFunctionType.Identity
    sub = mybir.AluOpType.subtract
    mult = mybir.AluOpType.mult
    add = mybir.AluOpType.add

    # dtype used for all matmul operands (f32r = full-rate fp32 stream mode;
    # tensors must be *typed* f32r end-to-end or the BIR verifier rejects)
    mdt = f32r if MM_MODE == "f32r" else f32

    nc = bacc.Bacc()
    x_d = nc.declare_dram_parameter("x", [nb, C, C], mdt, isOutput=False)
    dt_d = nc.declare_dram_parameter("dt", [C, C], mdt, isOutput=False)
    w1t_d = nc.declare_dram_parameter("w1t", [C, H], mdt, isOutput=False)
    b1_d = nc.declare_dram_parameter("b1", [H], f32, isOutput=False)
    w2t_d = nc.declare_dram_parameter("w2t", [H, C], mdt, isOutput=False)
    gb_d = nc.declare_dram_parameter("gb", [C, 2], f32, isOutput=False)
    out_d = nc.declare_dram_parameter("out", [nb, C, C], f32, isOutput=True)

    with TileContext(nc) as tc, \
            tc.tile_pool(name="consts", bufs=1) as consts, \
            tc.tile_pool(name="xin", bufs=3) as xin, \
            tc.tile_pool(name="work", bufs=2) as work, \
            tc.tile_pool(name="small", bufs=8) as small, \
            tc.tile_pool(name="res", bufs=4) as resp, \
            tc.tile_pool(name="ps", bufs=6, space="PSUM") as ps:

        dt_sb = consts.tile([P, KT, C], mdt)
        nc.sync.dma_start(out=dt_sb, in_=dt_d.rearrange("(t p) k -> p t k", p=P))
        w1t_sb = consts.tile([P, KT, H], mdt)
        nc.sync.dma_start(out=w1t_sb, in_=w1t_d.rearrange("(t p) h -> p t h", p=P))
        w2t_sb = consts.tile([P, HT, C], mdt)
        nc.sync.dma_start(out=w2t_sb, in_=w2t_d.rearrange("(t p) k -> p t k", p=P))
        b1_sb = consts.tile([P, HT], f32)
        nc.sync.dma_start(out=b1_sb, in_=b1_d.rearrange("(t p) -> p t", p=P))
        gb_sb = consts.tile([P, KT, 2], f32)
        nc.sync.dma_start(out=gb_sb, in_=gb_d.rearrange("(t p) g -> p t g", p=P))
        id_sb = consts.tile([P, P], f32)
        make_identity(nc, id_sb)
        eps_sb = consts.tile([P, 1], f32)
        nc.vector.memset(eps_sb, EPS)

        for b in range(nb):
            xb = xin.tile([P, KT, C], mdt, tag="xb")
            nc.sync.dma_start(out=xb, in_=x_d[b].rearrange("(t p) c -> p t c", p=P))

            # ---- DCT + LN1 (minus affine): z[c, k] ----
            z = work.tile([P, CT, C], f32, tag="z")
            for mc in range(CT):
                pf = ps.tile([P, C], f32, tag="ps")
                for lt in range(KT):
                    nc.tensor.matmul(
                        pf,
                        lhsT=xb[:, lt, mc * P:(mc + 1) * P],
                        rhs=dt_sb[:, lt, :],
                        start=(lt == 0),
                        stop=(lt == KT - 1),
                    )
                stats = small.tile([P, 6], f32, tag="stats")
                nc.vector.bn_stats(out=stats, in_=pf)
                mv = small.tile([P, 2], f32, tag="mv")
                nc.vector.bn_aggr(out=mv, in_=stats)
                lv = small.tile([P, 1], f32, tag="lv")
                nc.scalar.activation(out=lv, in_=mv[:, 1:2], func=Ln,
                                     bias=eps_sb, scale=1.0)
                rstd = small.tile([P, 1], f32, tag="rstd")
                nc.scalar.activation(out=rstd, in_=lv, func=Exp,
                                     bias=0.0, scale=-0.5)
                nmr = small.tile([P, 1], f32, tag="nmr")
                nc.vector.tensor_scalar(out=nmr, in0=mv[:, 0:1],
                                        scalar1=rstd, scalar2=-1.0,
                                        op0=mult, op1=mult)
                nc.scalar.activation(out=z[:, mc, :], in_=pf, func=Ident,
                                     bias=nmr, scale=rstd)

            # ---- transpose z -> zT [k, c] ----
            zT = work.tile([P, KT, C], mdt, tag="zT")
            for kt in range(KT):
                pt = ps.tile([P, C], f32, tag="ps")
                for mc in range(CT):
                    nc.tensor.transpose(pt[:, mc * P:(mc + 1) * P],
                                        z[:, mc, kt * P:(kt + 1) * P], id_sb)
                nc.scalar.copy(out=zT[:, kt, :], in_=pt)

            # ---- fc1: hT[h, c] = relu(W1g @ zT + b1) ----
            hT = work.tile([P, HT, C], mdt, tag="hT")
            for mh in range(HT):
                ph = ps.tile([P, C], f32, tag="ps")
                for kt in range(KT):
                    nc.tensor.matmul(
                        ph,
                        lhsT=w1t_sb[:, kt, mh * P:(mh + 1) * P],
                        rhs=zT[:, kt, :],
                        start=(kt == 0),
                        stop=(kt == KT - 1),
                    )
                nc.scalar.activation(out=hT[:, mh, :], in_=ph, func=Relu,
                                     bias=b1_sb[:, mh:mh + 1], scale=1.0)

            # ---- fc2 + sigmoid + LN2 (minus affine): z2[c, k] ----
            z2 = work.tile([P, CT, C], f32, tag="z2")
            for mc in range(CT):
                pw = ps.tile([P, C], f32, tag="ps")
                for ht in range(HT):
                    nc.tensor.matmul(
                        pw,
                        lhsT=hT[:, ht, mc * P:(mc + 1) * P],
                        rhs=w2t_sb[:, ht, :],
                        start=(ht == 0),
                        stop=(ht == HT - 1),
                    )
                et = work.tile([P, C], f32, tag="et")
                nc.scalar.activation(out=et, in_=pw, func=Exp,
                                     bias=0.0, scale=-1.0)
                nc.gpsimd.tensor_scalar_add(out=et, in0=et, scalar1=1.0)
                fwp = work.tile([P, C], f32, tag="fwp")
                nc.vector.reciprocal(out=fwp, in_=et)
                stats2 = small.tile([P, 6], f32, tag="stats")
                nc.vector.bn_stats(out=stats2, in_=fwp)
                mv2 = small.tile([P, 2], f32, tag="mv")
                nc.vector.bn_aggr(out=mv2, in_=stats2)
                lv2 = small.tile([P, 1], f32, tag="lv")
                nc.scalar.activation(out=lv2, in_=mv2[:, 1:2], func=Ln,
                                     bias=eps_sb, scale=1.0)
                rstd2 = small.tile([P, 1], f32, tag="rstd")
                nc.scalar.activation(out=rstd2, in_=lv2, func=Exp,
                                     bias=0.0, scale=-0.5)
                nmr2 = small.tile([P, 1], f32, tag="nmr")
                nc.vector.tensor_scalar(out=nmr2, in0=mv2[:, 0:1],
                                        scalar1=rstd2, scalar2=-1.0,
                                        op0=mult, op1=mult)
                nc.scalar.activation(out=z2[:, mc, :], in_=fwp, func=Ident,
                                     bias=nmr2, scale=rstd2)

            # ---- transpose z2, apply gamma/beta, multiply by x, store ----
            for kt in range(KT):
                pt2 = ps.tile([P, C], f32, tag="ps")
                for mc in range(CT):
                    nc.tensor.transpose(pt2[:, mc * P:(mc + 1) * P],
                                        z2[:, mc, kt * P:(kt + 1) * P], id_sb)
                res = resp.tile([P, C], f32, tag="res")
                nc.scalar.activation(out=res, in_=pt2, func=Ident,
                                     bias=gb_sb[:, kt, 1:2],
                                     scale=gb_sb[:, kt, 0:1])
                nc.vector.tensor_mul(out=res, in0=res, in1=xb[:, kt, :])
                nc.sync.dma_start(out=out_d[b, kt * P:(kt + 1) * P, :], in_=res)

    # Bacc's compile passes (register alloc, wait splitting for fp32 matmuls)
    # run in finalize(); the pjrt exec path requires a finalized module.
    nc.finalize()
    return nc


def get_nc(nb: int):
    key = (nb, MM_MODE)
    if key not in _NC_CACHE:
        _NC_CACHE[key] = _build(nb)
    return _NC_CACHE[key]


def make_host_inputs(x, gamma, beta, w1, w2):
    """Host-side precompute: DCT matrix + folded weights."""
    x = np.ascontiguousarray(np.asarray(x, dtype=np.float32))
    gamma = np.asarray(gamma, dtype=np.float32)
    beta = np.asarray(beta, dtype=np.float32)
    w1 = np.asarray(w1, dtype=np.float32)
    w2 = np.asarray(w2, dtype=np.float32)

    k = np.arange(C)[:, None].astype(np.float64)
    m = np.arange(C)[None, :].astype(np.float64)
    D = 2.0 * np.cos(np.pi * k * (2.0 * m + 1.0) / (2.0 * C))
    dt = np.ascontiguousarray(D.T.astype(np.float32))          # [l, k]
    w1t = np.ascontiguousarray((w1 * gamma[None, :]).T)        # [k, h]
    b1 = (w1 @ beta).astype(np.float32)                        # [h]
    w2t = np.ascontiguousarray(w2.T)                           # [h, k]
    gb = np.ascontiguousarray(np.stack([gamma, beta], axis=1))  # [k, 2]
    return x, dict(dt=dt, w1t=w1t, b1=b1, w2t=w2t, gb=gb)


def kernel(x, gamma, beta, w1, w2):
    from concourse.bass_utils import run_bass_kernel_spmd

    x, const = make_host_inputs(x, gamma, beta, w1, w2)
    nb = B_FULL // N_CORES
    nc = get_nc(nb)
    in_maps = [dict(x=x[i * nb:(i + 1) * nb], **const) for i in range(N_CORES)]
    r = run_bass_kernel_spmd(nc, in_maps, list(range(N_CORES)))
    return np.concatenate([r.results[i]["out"] for i in range(N_CORES)], axis=0)


# revision 8
# speedup vs baseline: 1.3051x; 1.3051x over previous
"""FECAM layer Trainium2 kernel.

Reference computation (per batch element b, X = x[b] in R^{512x512}, layout [l, c]):
    xp   = X^T                                  # [c, l]
    freq = xp @ D^T                             # DCT-II along l      [c, k]
    sd   = LN(freq) * gamma + beta              # LayerNorm over k
    h    = relu(sd @ W1^T)                      # [c, 2C]
    fw   = sigmoid(h @ W2^T)                    # [c, k]
    fw   = LN(fw) * gamma + beta
    out  = (xp * fw)^T = X .* fw^T              # [l, c]  (natural layout)

Device strategy (data parallel, 16 batch elements per core x 8 cores):
  - freq computed as matmul(lhsT=x_b_tiles [l,c], rhs=D^T tiles [l,k]) -> [c,k] psum
  - LN1 stats via bn_stats/bn_aggr (free-axis k), z=(freq-mu)*rstd via tensor_scalar
  - gamma/beta of LN1 folded into fc1 weights on host:
        W1g[h,k] = w1[h,k]*gamma[k],  b1[h] = sum_k beta[k]*w1[h,k]
  - z transposed 128x128 via PE into zT [k,c]; fc1: hT = relu(W1g @ zT + b1) in [h,c]
  - fc2: fw = sigmoid(hT^T @ W2^T) computed as matmul(lhsT=w2T cols, rhs=hT) -> [c,k]
  - LN2 same trick; affine applied after PE transpose as per-partition scale/bias
  - final: out_tile = (z2T*gamma+beta) .* x_tile, DMA'd out in natural layout
All matmuls in float32r (full fp32 precision, 1 cycle/row at free dim >= 256).
"""

import sys

if "/opt/trn_rl_repo" not in sys.path:
    sys.path.insert(0, "/opt/trn_rl_repo")

import numpy as np

P = 128
C = 512          # channels == seq len == dct size
H = 1024         # hidden
CT = C // P      # 4 c-tiles
KT = C // P      # 4 k-tiles
HT = H // P      # 8 h-tiles
EPS = 1e-6
N_CORES = 8
B_FULL = 128

_NC_CACHE: dict = {}

# matmul input dtype: "f32r" (fast, full fp32 bits) or "f32" (4x slower, safe)
MM_MODE = "f32r"


def _build(nb: int):
    import concourse.bass as bass
    from concourse import bacc
    import concourse.mybir as mybir
    from concourse.tile import TileContext
    from concourse.masks import make_identity

    f32 = mybir.dt.float32
    f32r = mybir.dt.float32r
    Relu = mybir.ActivationFunctionType.Relu
    Ln = mybir.ActivationFunctionType.Ln
    Exp = mybir.ActivationFunctionType.Exp
    Ident = mybir.ActivationFunctionType.Identity
    sub = mybir.AluOpType.subtract
    mult = mybir.AluOpType.mult
    add = mybir.AluOpType.add

    # dtype used for all matmul operands (f32r = full-rate fp32 stream mode;
    # tensors must be *typed* f32r end-to-end or the BIR verifier rejects)
    mdt = f32r if MM_MODE == "f32r" else f32

    nc = bacc.Bacc()
    x_d = nc.declare_dram_parameter("x", [nb, C, C], mdt, isOutput=False)
    dt_d = nc.declare_dram_parameter("dt", [C, C], mdt, isOutput=False)
    w1t_d = nc.declare_dram_parameter("w1t", [C, H], mdt, isOutput=False)
    b1_d = nc.declare_dram_parameter("b1", [H], f32, isOutput=False)
    w2t_d = nc.declare_dram_parameter("w2t", [H, C], mdt, isOutput=False)
    gb_d = nc.declare_dram_parameter("gb", [C, 2], f32, isOutput=False)
    out_d = nc.declare_dram_parameter("out", [nb, C, C], f32, isOutput=True)

    with TileContext(nc) as tc, \
            tc.tile_pool(name="consts", bufs=1) as consts, \
            tc.tile_pool(name="xin", bufs=3) as xin, \
            tc.tile_pool(name="work", bufs=2) as work, \
            tc.tile_pool(name="small", bufs=8) as small, \
            tc.tile_pool(name="res", bufs=4) as resp, \
            tc.tile_pool(name="ps", bufs=6, space="PSUM") as ps:

        dt_sb = consts.tile([P, KT, C], mdt)
        nc.sync.dma_start(out=dt_sb, in_=dt_d.rearrange("(t p) k -> p t k", p=P))
        w1t_sb = consts.tile([P, KT, H], mdt)
        nc.sync.dma_start(out=w1t_sb, in_=w1t_d.rearrange("(t p) h -> p t h", p=P))
        w2t_sb = consts.tile([P, HT, C], mdt)
        nc.sync.dma_start(out=w2t_sb, in_=w2t_d.rearrange("(t p) k -> p t k", p=P))
        b1_sb = consts.tile([P, HT], f32)
        nc.sync.dma_start(out=b1_sb, in_=b1_d.rearrange("(t p) -> p t", p=P))
        gb_sb = consts.tile([P, KT, 2], f32)
        nc.sync.dma_start(out=gb_sb, in_=gb_d.rearrange("(t p) g -> p t g", p=P))
        id_sb = consts.tile([P, P], f32)
        make_identity(nc, id_sb)
        eps_sb = consts.tile([P, 1], f32)
        nc.vector.memset(eps_sb, EPS)

        for b in range(nb):
            xb = xin.tile([P, KT, C], mdt, tag="xb")
            nc.sync.dma_start(out=xb, in_=x_d[b].rearrange("(t p) c -> p t c", p=P))

            # ---- DCT + LN1 (minus affine): z[c, k] ----
            z = work.tile([P, CT, C], f32, tag="z")
            for mc in range(CT):
                pf = ps.tile([P, C], f32, tag="ps")
                for lt in range(KT):
                    nc.tensor.matmul(
                        pf,
                        lhsT=xb[:, lt, mc * P:(mc + 1) * P],
                        rhs=dt_sb[:, lt, :],
                        start=(lt == 0),
                        stop=(lt == KT - 1),
                    )
                stats = small.tile([P, 6], f32, tag="stats")
                nc.vector.bn_stats(out=stats, in_=pf)
                mv = small.tile([P, 2], f32, tag="mv")
                nc.vector.bn_aggr(out=mv, in_=stats)
                lv = small.tile([P, 1], f32, tag="lv")
                nc.scalar.activation(out=lv, in_=mv[:, 1:2], func=Ln,
                                     bias=eps_sb, scale=1.0)
                rstd = small.tile([P, 1], f32, tag="rstd")
                nc.scalar.activation(out=rstd, in_=lv, func=Exp,
                                     bias=0.0, scale=-0.5)
                nmr = small.tile([P, 1], f32, tag="nmr")
                nc.vector.tensor_scalar(out=nmr, in0=mv[:, 0:1],
                                        scalar1=rstd, scalar2=-1.0,
                                        op0=mult, op1=mult)
                nc.scalar.activation(out=z[:, mc, :], in_=pf, func=Ident,
                                     bias=nmr, scale=rstd)

            # ---- transpose z -> zT [k, c] ----
            zT = work.tile([P, KT, C], mdt, tag="zT")
            for kt in range(KT):
                pt = ps.tile([P, C], f32, tag="ps")
                for mc in range(CT):
                    nc.tensor.transpose(pt[:, mc * P:(mc + 1) * P],
                                        z[:, mc, kt * P:(kt + 1) * P], id_sb)
                nc.scalar.copy(out=zT[:, kt, :], in_=pt)

            # ---- fc1: hT[h, c] = relu(W1g @ zT + b1) ----
            hT = work.tile([P, HT, C], mdt, tag="hT")
            for mh in range(HT):
                ph = ps.tile([P, C], f32, tag="ps")
                for kt in range(KT):
                    nc.tensor.matmul(
                        ph,
                        lhsT=w1t_sb[:, kt, mh * P:(mh + 1) * P],
                        rhs=zT[:, kt, :],
                        start=(kt == 0),
                        stop=(kt == KT - 1),
                    )
                nc.scalar.activation(out=hT[:, mh, :], in_=ph, func=Relu,
                                     bias=b1_sb[:, mh:mh + 1], scale=1.0)

            # ---- fc2 + sigmoid + LN2 (minus affine): z2[c, k] ----
            z2 = work.tile([P, CT, C], f32, tag="z2")
            for mc in range(CT):
                pw = ps.tile([P, C], f32, tag="ps")
                for ht in range(HT):
                    nc.tensor.matmul(
                        pw,
                        lhsT=hT[:, ht, mc * P:(mc + 1) * P],
                        rhs=w2t_sb[:, ht, :],
                        start=(ht == 0),
                        stop=(ht == HT - 1),
                    )
                et = work.tile([P, C], f32, tag="et")
                nc.scalar.activation(out=et, in_=pw, func=Exp,
                                     bias=0.0, scale=-1.0)
                nc.vector.tensor_scalar_add(out=et, in0=et, scalar1=1.0)
                fwp = work.tile([P, C], f32, tag="fwp")
                nc.vector.reciprocal(out=fwp, in_=et)
                stats2 = small.tile([P, 6], f32, tag="stats")
                nc.vector.bn_stats(out=stats2, in_=fwp)
                mv2 = small.tile([P, 2], f32, tag="mv")
                nc.vector.bn_aggr(out=mv2, in_=stats2)
                lv2 = small.tile([P, 1], f32, tag="lv")
                nc.scalar.activation(out=lv2, in_=mv2[:, 1:2], func=Ln,
                                     bias=eps_sb, scale=1.0)
                rstd2 = small.tile([P, 1], f32, tag="rstd")
                nc.scalar.activation(out=rstd2, in_=lv2, func=Exp,
                                     bias=0.0, scale=-0.5)
                nmr2 = small.tile([P, 1], f32, tag="nmr")
                nc.vector.tensor_scalar(out=nmr2, in0=mv2[:, 0:1],
                                        scalar1=rstd2, scalar2=-1.0,
                                        op0=mult, op1=mult)
                nc.scalar.activation(out=z2[:, mc, :], in_=fwp, func=Ident,
                                     bias=nmr2, scale=rstd2)

            # ---- transpose z2, apply gamma/beta, multiply by x, store ----
            for kt in range(KT):
                pt2 = ps.tile([P, C], f32, tag="ps")
                for mc in range(CT):
                    nc.tensor.transpose(pt2[:, mc * P:(mc + 1) * P],
                                        z2[:, mc, kt * P:(kt + 1) * P], id_sb)
                res = resp.tile([P, C], f32, tag="res")
                nc.scalar.activation(out=res, in_=pt2, func=Ident,
                                     bias=gb_sb[:, kt, 1:2],
                                     scale=gb_sb[:, kt, 0:1])
                nc.vector.tensor_mul(out=res, in0=res, in1=xb[:, kt, :])
                nc.sync.dma_start(out=out_d[b, kt * P:(kt + 1) * P, :], in_=res)

    # Bacc's compile passes (register alloc, wait splitting for fp32 matmuls)
    # run in finalize(); the pjrt exec path requires a finalized module.
    nc.finalize()
    return nc


def get_nc(nb: int):
    key = (nb, MM_MODE)
    if key not in _NC_CACHE:
        _NC_CACHE[key] = _build(nb)
    return _NC_CACHE[key]


def make_host_inputs(x, gamma, beta, w1, w2):
    """Host-side precompute: DCT matrix + folded weights."""
    x = np.ascontiguousarray(np.asarray(x, dtype=np.float32))
    gamma = np.asarray(gamma, dtype=np.float32)
    beta = np.asarray(beta, dtype=np.float32)
    w1 = np.asarray(w1, dtype=np.float32)
    w2 = np.asarray(w2, dtype=np.float32)

    k = np.arange(C)[:, None].astype(np.float64)
    m = np.arange(C)[None, :].astype(np.float64)
    D = 2.0 * np.cos(np.pi * k * (2.0 * m + 1.0) / (2.0 * C))
    dt = np.ascontiguousarray(D.T.astype(np.float32))          # [l, k]
    w1t = np.ascontiguousarray((w1 * gamma[None, :]).T)        # [k, h]
    b1 = (w1 @ beta).astype(np.float32)                        # [h]
    w2t = np.ascontiguousarray(w2.T)                           # [h, k]
    gb = np.ascontiguousarray(np.stack([gamma, beta], axis=1))  # [k, 2]
    return x, dict(dt=dt, w1t=w1t, b1=b1, w2t=w2t, gb=gb)


def kernel(x, gamma, beta, w1, w2):
    from concourse.bass_utils import run_bass_kernel_spmd

    x, const = make_host_inputs(x, gamma, beta, w1, w2)
    nb = B_FULL // N_CORES
    nc = get_nc(nb)
    in_maps = [dict(x=x[i * nb:(i + 1) * nb], **const) for i in range(N_CORES)]
    r = run_bass_kernel_spmd(nc, in_maps, list(range(N_CORES)))
    return np.concatenate([r.results[i]["out"] for i in range(N_CORES)], axis=0)


# revision 9
# speedup vs baseline: 2.0186x; 1.5466x over previous
"""FECAM layer Trainium2 kernel.

Reference computation (per batch element b, X = x[b] in R^{512x512}, layout [l, c]):
    xp   = X^T                                  # [c, l]
    freq = xp @ D^T                             # DCT-II along l      [c, k]
    sd   = LN(freq) * gamma + beta              # LayerNorm over k
    h    = relu(sd @ W1^T)                      # [c, 2C]
    fw   = sigmoid(h @ W2^T)                    # [c, k]
    fw   = LN(fw) * gamma + beta
    out  = (xp * fw)^T = X .* fw^T              # [l, c]  (natural layout)

Device strategy (data parallel, 16 batch elements per core x 8 cores):
  - freq computed as matmul(lhsT=x_b_tiles [l,c], rhs=D^T tiles [l,k]) -> [c,k] psum
  - LN1 stats via bn_stats/bn_aggr (free-axis k), z=(freq-mu)*rstd via tensor_scalar
  - gamma/beta of LN1 folded into fc1 weights on host:
        W1g[h,k] = w1[h,k]*gamma[k],  b1[h] = sum_k beta[k]*w1[h,k]
  - z transposed 128x128 via PE into zT [k,c]; fc1: hT = relu(W1g @ zT + b1) in [h,c]
  - fc2: fw = sigmoid(hT^T @ W2^T) computed as matmul(lhsT=w2T cols, rhs=hT) -> [c,k]
  - LN2 same trick; affine applied after PE transpose as per-partition scale/bias
  - final: out_tile = (z2T*gamma+beta) .* x_tile, DMA'd out in natural layout
All matmuls in float32r (full fp32 precision, 1 cycle/row at free dim >= 256).
"""

import sys

if "/opt/trn_rl_repo" not in sys.path:
    sys.path.insert(0, "/opt/trn_rl_repo")

import numpy as np

P = 128
C = 512          # channels == seq len == dct size
H = 1024         # hidden
CT = C // P      # 4 c-tiles
KT = C // P      # 4 k-tiles
HT = H // P      # 8 h-tiles
EPS = 1e-6
N_CORES = 8
B_FULL = 128

_NC_CACHE: dict = {}

# matmul input dtype: "f32r" (fast, full fp32 bits) or "f32" (4x slower, safe)
MM_MODE = "f32r"


def _build(nb: int):
    import concourse.bass as bass
    from concourse import bacc
    import concourse.mybir as mybir
    from concourse.tile import TileContext
    from concourse.masks import make_identity

    f32 = mybir.dt.float32
    f32r = mybir.dt.float32r
    Relu = mybir.ActivationFunctionType.Relu
    Ln = mybir.ActivationFunctionType.Ln
    Exp = mybir.ActivationFunctionType.Exp
    Ident = mybir.ActivationFunctionType.Identity
    sub = mybir.AluOpType.subtract
    mult = mybir.AluOpType.mult
    add = mybir.AluOpType.add

    # dtype used for all matmul operands (f32r = full-rate fp32 stream mode;
    # tensors must be *typed* f32r end-to-end or the BIR verifier rejects)
    mdt = f32r if MM_MODE == "f32r" else f32

    nc = bacc.Bacc()
    x_d = nc.declare_dram_parameter("x", [nb, C, C], mdt, isOutput=False)
    dt_d = nc.declare_dram_parameter("dt", [C, C], mdt, isOutput=False)
    w1t_d = nc.declare_dram_parameter("w1t", [C, H], mdt, isOutput=False)
    b1_d = nc.declare_dram_parameter("b1", [H], f32, isOutput=False)
    w2t_d = nc.declare_dram_parameter("w2t", [H, C], mdt, isOutput=False)
    gb_d = nc.declare_dram_parameter("gb", [C, 2], f32, isOutput=False)
    out_d = nc.declare_dram_parameter("out", [nb, C, C], f32, isOutput=True)

    with TileContext(nc) as tc, \
            tc.tile_pool(name="consts", bufs=1) as consts, \
            tc.tile_pool(name="xin", bufs=3) as xin, \
            tc.tile_pool(name="work", bufs=2) as work, \
            tc.tile_pool(name="small", bufs=8) as small, \
            tc.tile_pool(name="res", bufs=4) as resp, \
            tc.tile_pool(name="ps", bufs=6, space="PSUM") as ps:

        from concourse.hw_specs import get_activation_tables
        set_names = list(get_activation_tables(nc.m.arch))
        nc.scalar.add_instruction(mybir.InstLoadActFuncSet(
            name=nc.get_next_instruction_name(),
            act_func_set_id=set_names.index("natural_log_exp_and_others"),
            ins=[], outs=[]))

        dt_sb = consts.tile([P, KT, C], mdt)
        nc.sync.dma_start(out=dt_sb, in_=dt_d.rearrange("(t p) k -> p t k", p=P))
        w1t_sb = consts.tile([P, KT, H], mdt)
        nc.sync.dma_start(out=w1t_sb, in_=w1t_d.rearrange("(t p) h -> p t h", p=P))
        w2t_sb = consts.tile([P, HT, C], mdt)
        nc.sync.dma_start(out=w2t_sb, in_=w2t_d.rearrange("(t p) k -> p t k", p=P))
        b1_sb = consts.tile([P, HT], f32)
        nc.sync.dma_start(out=b1_sb, in_=b1_d.rearrange("(t p) -> p t", p=P))
        gb_sb = consts.tile([P, KT, 2], f32)
        nc.sync.dma_start(out=gb_sb, in_=gb_d.rearrange("(t p) g -> p t g", p=P))
        id_sb = consts.tile([P, P], f32)
        make_identity(nc, id_sb)
        eps_sb = consts.tile([P, 1], f32)
        nc.vector.memset(eps_sb, EPS)

        for b in range(nb):
            xb = xin.tile([P, KT, C], mdt, tag="xb")
            nc.sync.dma_start(out=xb, in_=x_d[b].rearrange("(t p) c -> p t c", p=P))

            # ---- DCT + LN1 (minus affine): z[c, k] ----
            z = work.tile([P, CT, C], f32, tag="z")
            for mc in range(CT):
                pf = ps.tile([P, C], f32, tag="ps")
                for lt in range(KT):
                    nc.tensor.matmul(
                        pf,
                        lhsT=xb[:, lt, mc * P:(mc + 1) * P],
                        rhs=dt_sb[:, lt, :],
                        start=(lt == 0),
                        stop=(lt == KT - 1),
                    )
                stats = small.tile([P, 6], f32, tag="stats")
                nc.vector.bn_stats(out=stats, in_=pf)
                mv = small.tile([P, 2], f32, tag="mv")
                nc.vector.bn_aggr(out=mv, in_=stats)
                lv = small.tile([P, 1], f32, tag="lv")
                nc.scalar.activation(out=lv, in_=mv[:, 1:2], func=Ln,
                                     bias=eps_sb, scale=1.0)
                rstd = small.tile([P, 1], f32, tag="rstd")
                nc.scalar.activation(out=rstd, in_=lv, func=Exp,
                                     bias=0.0, scale=-0.5)
                nmr = small.tile([P, 1], f32, tag="nmr")
                nc.vector.tensor_scalar(out=nmr, in0=mv[:, 0:1],
                                        scalar1=rstd, scalar2=-1.0,
                                        op0=mult, op1=mult)
                nc.scalar.activation(out=z[:, mc, :], in_=pf, func=Ident,
                                     bias=nmr, scale=rstd)

            # ---- transpose z -> zT [k, c] ----
            zT = work.tile([P, KT, C], mdt, tag="zT")
            for kt in range(KT):
                pt = ps.tile([P, C], f32, tag="ps")
                for mc in range(CT):
                    nc.tensor.transpose(pt[:, mc * P:(mc + 1) * P],
                                        z[:, mc, kt * P:(kt + 1) * P], id_sb)
                nc.scalar.copy(out=zT[:, kt, :], in_=pt)

            # ---- fc1: hT[h, c] = relu(W1g @ zT + b1) ----
            hT = work.tile([P, HT, C], mdt, tag="hT")
            for mh in range(HT):
                ph = ps.tile([P, C], f32, tag="ps")
                for kt in range(KT):
                    nc.tensor.matmul(
                        ph,
                        lhsT=w1t_sb[:, kt, mh * P:(mh + 1) * P],
                        rhs=zT[:, kt, :],
                        start=(kt == 0),
                        stop=(kt == KT - 1),
                    )
                nc.scalar.activation(out=hT[:, mh, :], in_=ph, func=Relu,
                                     bias=b1_sb[:, mh:mh + 1], scale=1.0)

            # ---- fc2 + sigmoid + LN2 (minus affine): z2[c, k] ----
            z2 = work.tile([P, CT, C], f32, tag="z2")
            for mc in range(CT):
                pw = ps.tile([P, C], f32, tag="ps")
                for ht in range(HT):
                    nc.tensor.matmul(
                        pw,
                        lhsT=hT[:, ht, mc * P:(mc + 1) * P],
                        rhs=w2t_sb[:, ht, :],
                        start=(ht == 0),
                        stop=(ht == HT - 1),
                    )
                et = work.tile([P, C], f32, tag="et")
                nc.scalar.activation(out=et, in_=pw, func=Exp,
                                     bias=0.0, scale=-1.0)
                nc.vector.tensor_scalar_add(out=et, in0=et, scalar1=1.0)
                fwp = work.tile([P, C], f32, tag="fwp")
                nc.vector.reciprocal_approx_fast(out=fwp, in_=et)
                stats2 = small.tile([P, 6], f32, tag="stats")
                nc.vector.bn_stats(out=stats2, in_=fwp)
                mv2 = small.tile([P, 2], f32, tag="mv")
                nc.vector.bn_aggr(out=mv2, in_=stats2)
                lv2 = small.tile([P, 1], f32, tag="lv")
                nc.scalar.activation(out=lv2, in_=mv2[:, 1:2], func=Ln,
                                     bias=eps_sb, scale=1.0)
                rstd2 = small.tile([P, 1], f32, tag="rstd")
                nc.scalar.activation(out=rstd2, in_=lv2, func=Exp,
                                     bias=0.0, scale=-0.5)
                nmr2 = small.tile([P, 1], f32, tag="nmr")
                nc.vector.tensor_scalar(out=nmr2, in0=mv2[:, 0:1],
                                        scalar1=rstd2, scalar2=-1.0,
                                        op0=mult, op1=mult)
                nc.scalar.activation(out=z2[:, mc, :], in_=fwp, func=Ident,
                                     bias=nmr2, scale=rstd2)

            # ---- transpose z2, apply gamma/beta, multiply by x, store ----
            for kt in range(KT):
                pt2 = ps.tile([P, C], f32, tag="ps")
                for mc in range(CT):
                    nc.tensor.transpose(pt2[:, mc * P:(mc + 1) * P],
                                        z2[:, mc, kt * P:(kt + 1) * P], id_sb)
                res = resp.tile([P, C], f32, tag="res")
                nc.scalar.activation(out=res, in_=pt2, func=Ident,
                                     bias=gb_sb[:, kt, 1:2],
                                     scale=gb_sb[:, kt, 0:1])
                nc.vector.tensor_mul(out=res, in0=res, in1=xb[:, kt, :])
                nc.sync.dma_start(out=out_d[b, kt * P:(kt + 1) * P, :], in_=res)

    # Bacc's compile passes (register alloc, wait splitting for fp32 matmuls)
    # run in finalize(); the pjrt exec path requires a finalized module.
    nc.finalize()
    return nc


def get_nc(nb: int):
    key = (nb, MM_MODE)
    if key not in _NC_CACHE:
        _NC_CACHE[key] = _build(nb)
    return _NC_CACHE[key]


def make_host_inputs(x, gamma, beta, w1, w2):
    """Host-side precompute: DCT matrix + folded weights."""
    x = np.ascontiguousarray(np.asarray(x, dtype=np.float32))
    gamma = np.asarray(gamma, dtype=np.float32)
    beta = np.asarray(beta, dtype=np.float32)
    w1 = np.asarray(w1, dtype=np.float32)
    w2 = np.asarray(w2, dtype=np.float32)

    k = np.arange(C)[:, None].astype(np.float64)
    m = np.arange(C)[None, :].astype(np.float64)
    D = 2.0 * np.cos(np.pi * k * (2.0 * m + 1.0) / (2.0 * C))
    dt = np.ascontiguousarray(D.T.astype(np.float32))          # [l, k]
    w1t = np.ascontiguousarray((w1 * gamma[None, :]).T)        # [k, h]
    b1 = (w1 @ beta).astype(np.float32)                        # [h]
    w2t = np.ascontiguousarray(w2.T)                           # [h, k]
    gb = np.ascontiguousarray(np.stack([gamma, beta], axis=1))  # [k, 2]
    return x, dict(dt=dt, w1t=w1t, b1=b1, w2t=w2t, gb=gb)


def kernel(x, gamma, beta, w1, w2):
    from concourse.bass_utils import run_bass_kernel_spmd

    x, const = make_host_inputs(x, gamma, beta, w1, w2)
    nb = B_FULL // N_CORES
    nc = get_nc(nb)
    in_maps = [dict(x=x[i * nb:(i + 1) * nb], **const) for i in range(N_CORES)]
    r = run_bass_kernel_spmd(nc, in_maps, list(range(N_CORES)))
    return np.concatenate([r.results[i]["out"] for i in range(N_CORES)], axis=0)
